# revision 6
# baseline (speedup 1.0000x reference)
"""Trainium2 Bass kernel for nn_DowngradeProtocol (mtf blur + fineshift + 4x decimate).

F9 restructure: the baseline was LDWEIGHTS-bound (each of its 2880 matmuls/core
reloaded a [128,128] image stationary for only 27 moving rows; 154us ~= pure
weight-load time). F9 eliminates the 9x stationary reloads:

  stage A: per (row-window g, col-chunk cc): ONE matmul with the unshifted
           image window as stationary and all 9 mtf-column weight sets as
           243 moving rows -> 9 uncombined planes t_v[c, y'] in PSUM.
  stage B: contracts the 9 planes over image columns with v-shifted
           horizontal (fineshift+decimate) banded kernels, accumulating all
           (tile, v) contributions in PSUM -> out[y, x'] directly.

Exact algebraic refactor of the same banded matrices (no approximation).
"""
import sys

import numpy as np

for _p in ("/opt/trn_rl_repo",):
    if _p not in sys.path:
        sys.path.insert(0, _p)

# ---------------------------------------------------------------- constants
H = W = 1024
OUT = 256
NG = 10       # row windows of 128 (stride 108); 27 sampled rows each
GRP = 27
NV = 9        # mtf horizontal offsets
TW = NG * GRP  # 270: per-v y' width of a t-plane column block
N_CORES = 8
IMG_PER_CORE = 4

_HALF = np.asarray([0.5, 0.305334091185, 0, -0.072698593239, 0, 0.021809577942,
                    0, -0.005192756653, 0, 0.000807762146, 0, -6.0081482e-05]) * 2.0
_FULL23 = np.concatenate([_HALF[1:][::-1], _HALF])
F12 = _FULL23[::2]
DELTA12 = np.zeros(12)
DELTA12[6] = 1.0


def _wb(g):
    return 108 * g - 10


# ------------------------------------------------------- host weight builders
def build_BV(m2d, rr):
    """Vertical banded matrices [NG, 9, 128, GRP] f64 for one image.

    BV[g, v, k, n] = weight of input row (wb(g)+k) for sampled output row
    y'=27g+n under mtf column offset v, with the fine-shift vertical kernel
    fused in, input-edge replication folded (row clip), and fine-stage
    zero-padding honored.
    """
    ri, rf = rr // 2, rr % 2
    f = F12 if rf == 1 else DELTA12
    BV = np.zeros((NG, 9, 128, GRP))
    for g in range(NG):
        wb = _wb(g)
        for n in range(GRP):
            yp = 27 * g + n
            if yp >= OUT:
                continue
            Ry = 2 + 4 * yp - ri
            for up in range(12):
                i1 = Ry + up - 6
                fw = f[up]
                if fw == 0.0 or not (0 <= i1 < H):
                    continue
                for u in range(9):
                    k = min(max(i1 + u - 4, 0), H - 1)
                    BV[g, :, k - wb, n] += fw * m2d[u, :]
    return BV


def _khv_geometry():
    """Static stage-B block table: for each (v, tile t) the x'-range whose
    12-tap horizontal band (shifted by v, edge-clipped) intersects z-column
    tile t, unioned over ci in {0,1,2} so the program is image-independent.
    Returns (blocks=[(v, t, x0, x1, koff)], KHW)."""
    nz = np.zeros((NV, 8, OUT), bool)
    for ci in range(3):
        for x in range(OUT):
            Cx = 2 + 4 * x - ci
            for tt in range(12):
                jz = Cx + tt - 6
                if 0 <= jz < W:
                    for v in range(NV):
                        jx = min(max(jz + v - 4, 0), W - 1)
                        nz[v, jx // 128, x] = True
    blocks = []
    off = 0
    for v in range(NV):
        for t in range(8):
            xs = np.nonzero(nz[v, t])[0]
            if len(xs) == 0:
                continue
            x0, x1 = int(xs[0]), int(xs[-1]) + 1
            assert np.all(nz[v, t, x0:x1]), (v, t)
            blocks.append((v, t, x0, x1, off))
            off += x1 - x0
    return blocks, off


BLOCKS, KHW = _khv_geometry()


def build_KHV(cc_val):
    """Per-image stage-B data [128, KHW] f64 filled into the static blocks."""
    ci, cf = cc_val // 2, cc_val % 2
    h = F12 if cf == 1 else DELTA12
    F = np.zeros((NV, W, OUT))
    for x in range(OUT):
        Cx = 2 + 4 * x - ci
        for tt in range(12):
            jz = Cx + tt - 6
            if not (0 <= jz < W):
                continue
            hv = h[tt]
            if hv == 0.0:
                continue
            for v in range(NV):
                jx = min(max(jz + v - 4, 0), W - 1)
                F[v, jx, x] += hv
    kh = np.zeros((128, KHW))
    for (v, t, x0, x1, off) in BLOCKS:
        kh[:, off:off + (x1 - x0)] = F[v, 128 * t:128 * (t + 1), x0:x1]
    return kh


# ------------------------------------------------------------- bass program
_PROGRAM = None


def _split_multi_waits(nc):
    """This container's walrus codegen allows only ONE sync-wait per
    instruction; hoist extra waits onto NoOps inserted just before, on the
    same engine (engine blocks on each in program order — semantics kept)."""
    import concourse.mybir as mybir

    n_split = 0
    for fn in nc.m.functions:
        for bb in fn.blocks:
            out = []
            changed = False
            for inst in bb.instructions:
                si = getattr(inst, "sync_info", None)
                waits = list(si.on_wait) if si is not None and si.on_wait else []
                if len(waits) > 1:
                    for w in waits[:-1]:
                        nop = mybir.InstNoOp(
                            text_hint="wait_split",
                            name=f"I-{nc.next_id()}",
                            engine=inst.engine,
                            ins=[], outs=[],
                            sync_info=mybir.SyncInfo(on_wait=[w], on_update=[]),
                        )
                        nc.register_instruction(nop)
                        out.append(nop)
                        n_split += 1
                    si.on_wait[:] = waits[-1:]
                    changed = True
                out.append(inst)
            if changed:
                bb.instructions[:] = out
    return n_split


def _build_program():
    import concourse.bass as bass
    import concourse.mybir as mybir
    from concourse.tile import TileContext

    f32, f16 = mybir.dt.float32, mybir.dt.float16
    nc = bass.Bass(target_bir_lowering=False, trn_type="TRN2")

    bvw = NG * NV * GRP
    x_in = nc.dram_tensor("x", [IMG_PER_CORE, 128, NG, W], f16,
                          kind="ExternalInput")
    w_in = nc.dram_tensor("w", [128, IMG_PER_CORE * (bvw + KHW)], f16,
                          kind="ExternalInput")
    out_t = nc.dram_tensor("out", [IMG_PER_CORE, OUT, OUT], f16,
                           kind="ExternalOutput")

    with TileContext(nc) as tc:
        with (
            tc.tile_pool(name="pw", bufs=2) as pw,
            tc.tile_pool(name="pxe", bufs=2) as pxe,
            tc.tile_pool(name="pt", bufs=2) as pt,
            tc.tile_pool(name="pout", bufs=3) as pout,
            tc.tile_pool(name="psA", bufs=6, space="PSUM") as psA,
            tc.tile_pool(name="psB", bufs=2, space="PSUM") as psB,
        ):
            zt = pw.tile([128, OUT], f16, tag="zt")
            nc.vector.memset(zt[:, :], 0.0)
            # warm-up: PE work that depends only on the memset, filling the
            # initial DMA-fill idle and ramping the p-state before real data
            # arrives (results never read).
            wu = psB.tile([128, OUT], f32, tag="psB", name="wu")
            for _ in range(32):
                nc.tensor.matmul(wu[:, :], lhsT=zt[:, 0:128], rhs=zt[:, :],
                                 start=True, stop=True)
            ncopy = 0
            for img in range(IMG_PER_CORE):
                # ---- batched loads: bv, windows g0-3, windows g4-9, kh.
                # Few big DMA triggers — SP-queue serialization was the
                # dominant stall with per-window DMAs.
                w_sb = pw.tile([128, bvw + KHW], f16, tag="w")
                woff = img * (bvw + KHW)
                nc.sync.dma_start(out=w_sb[:, 0:bvw],
                                  in_=w_in[:, woff:woff + bvw])
                xa = pxe.tile([128, NG, W], f16, tag="xa")
                nc.sync.dma_start(out=xa[:, 0:4, :],
                                  in_=x_in[img, :, 0:4, :])
                nc.sync.dma_start(out=xa[:, 4:NG, :],
                                  in_=x_in[img, :, 4:NG, :])
                nc.sync.dma_start(out=w_sb[:, bvw:bvw + KHW],
                                  in_=w_in[:, woff + bvw:woff + bvw + KHW])
                bv_sb = w_sb
                kh0 = bvw

                # ---- stage A: one matmul per (g, cc); 9 t_v planes at once.
                # g-pair outer so compute starts once the first windows land.
                tpl = {cc: pt.tile([128, NV, TW], f16, tag=f"T{cc}",
                                   name=f"T{cc}")
                       for cc in range(8)}
                for gp in range(5):
                    for cc in range(8):
                        ps = psA.tile([128, 2, NV, GRP], f32, tag="psA")
                        for gi in range(2):
                            g = 2 * gp + gi
                            off = g * NV * GRP
                            nc.tensor.matmul(
                                ps[:, gi],
                                lhsT=xa[:, g, 128 * cc:128 * (cc + 1)],
                                rhs=bv_sb[:, off:off + NV * GRP],
                                start=True, stop=True)
                        # T[:, v, 54*gp + 27*gi + n] = ps[:, gi, v, n]
                        src = ps[:].transpose([0, 2, 1, 3])
                        dst = tpl[cc][:, :, 54 * gp:54 * gp + 54]
                        if ncopy % 2 == 0:
                            nc.vector.tensor_copy(out=dst, in_=src)
                        else:
                            nc.scalar.copy(out=dst, in_=src)
                        ncopy += 1

                # ---- stage B: accumulate all (tile, v) blocks into out PSUM
                for yc in range(2):
                    po = psB.tile([128, OUT], f32, tag="psB")
                    # zero + set PSUM written-bits via an all-zero matmul
                    nc.tensor.matmul(
                        po[:, :],
                        lhsT=tpl[0][:, 0, 0:128],
                        rhs=zt[:, :],
                        start=True, stop=False, skip_group_check=True)
                    for bi, (v, t, x0, x1, koff) in enumerate(BLOCKS):
                        nc.tensor.matmul(
                            po[:, x0:x1],
                            lhsT=tpl[t][:, v, 128 * yc:128 * yc + 128],
                            rhs=w_sb[:, kh0 + koff:kh0 + koff + (x1 - x0)],
                            start=False, stop=(bi == len(BLOCKS) - 1),
                            skip_group_check=True)
                    ot = pout.tile([128, OUT], f16, tag="ot")
                    if yc == 0:
                        nc.vector.tensor_copy(out=ot[:, :], in_=po[:, :])
                    else:
                        nc.scalar.copy(out=ot[:, :], in_=po[:, :])
                    nc.sync.dma_start(
                        out=out_t[img, 128 * yc:128 * (yc + 1), :],
                        in_=ot[:, :])
    _split_multi_waits(nc)
    return nc


def _get_program():
    global _PROGRAM
    if _PROGRAM is None:
        _PROGRAM = _build_program()
    return _PROGRAM


# ------------------------------------------------------------------ kernel
def _window_image(img16):
    """[NG, 128, W] f16 pre-extracted row windows; out-of-image partitions
    get arbitrary finite rows (their BV weights are exactly zero)."""
    xw = np.empty((NG, 128, W), np.float16)
    for g in range(NG):
        wb = _wb(g)
        r0, r1 = max(0, wb), min(H, wb + 128)
        p0, pn = r0 - wb, r1 - r0
        xw[g, p0:p0 + pn] = img16[r0:r1]
        if p0 > 0:
            xw[g, 0:p0] = img16[0:p0]
        if p0 + pn < 128:
            xw[g, p0 + pn:] = img16[0:128 - (p0 + pn)]
    return xw


def _make_in_maps(outputs, mtf, r, c):
    outputs = np.ascontiguousarray(outputs, np.float32)
    mtf64 = np.asarray(mtf, np.float64)
    bvw = NG * NV * GRP
    in_maps = []
    for core in range(N_CORES):
        xs = np.empty((IMG_PER_CORE, 128, NG, W), np.float16)
        w = np.empty((128, IMG_PER_CORE, bvw + KHW), np.float16)
        for i in range(IMG_PER_CORE):
            p = core * IMG_PER_CORE + i
            b, ch = divmod(p, 8)
            xs[i] = _window_image(
                outputs[b, ch].astype(np.float16)).transpose(1, 0, 2)
            BV = build_BV(mtf64[:, :, ch], int(r[b, ch]))   # [NG,9,128,GRP]
            w[:, i, 0:bvw] = BV.transpose(2, 0, 1, 3).reshape(
                128, bvw).astype(np.float16)
            w[:, i, bvw:] = build_KHV(int(c[b, ch])).astype(np.float16)
        in_maps.append({
            "x": xs,
            "w": np.ascontiguousarray(w.reshape(128, -1)),
        })
    return in_maps


def run(outputs, mtf, r, c, trace=False, trace_cores=None):
    from concourse.bass_utils import run_bass_kernel_spmd

    nc = _get_program()
    in_maps = _make_in_maps(outputs, mtf, r, c)
    res = run_bass_kernel_spmd(nc, in_maps, core_ids=list(range(N_CORES)),
                               trace=trace, trace_cores=trace_cores)
    full = np.empty((4, 8, OUT, OUT), np.float32)
    for core in range(N_CORES):
        o = np.asarray(res.results[core]["out"])
        for i in range(IMG_PER_CORE):
            p = core * IMG_PER_CORE + i
            b, ch = divmod(p, 8)
            full[b, ch] = o[i].astype(np.float32)
    return full, res


def kernel(outputs, mtf, r, c):
    full, _ = run(outputs, mtf, r, c)
    return full
